# revision 43
# baseline (speedup 1.0000x reference)
"""Based linear-attention (parallel form) on 8 TRN2 NeuronCores.

Sharding: core c handles batch b = c // 4 and head-group g = c % 4
(3 of 12 heads).  Wq/Wk/Wv are column-split by head, Wo row-split; each
core emits a partial transposed [D, L] output and the host transposes
and sums the 4 partials per batch.

Device algorithm per core (all matmuls contract on the partition dim):
  hsT [D, L] (host-transposed, bf16) arrives as 48 host-packed
  contiguous [128, 512] chunks spread over the sync/gpsimd/scalar DMA
  queues (fat descriptors, parallel rings); weights are host-packed
  into SBUF layout so each loads with a single descriptor-efficient
  DMA.  A PE warmup loop runs during DMA priming to hold the tensor
  engine's DVFS ramp at max clock.

  q+k projection is one compact 12-matmul chain per 512-l-strip
  (M=96, all columns useful: [q0 q1 q2 k0 k1 k2] x 16), then two 0/1
  selection matmuls spread rows to the 32-aligned qT/kT layout that
  the PE's quadrant rules require.  v [L, 384] per-m-tile chains.

  Attention per l-strip of 512, per m-tile, per head: sT = k q^T
  (K=16, three heads in distinct 32-row PE groups run concurrently),
  attnT = (sT+1)^2 on ACT (+1 and causal mask on DVE; the mask STT
  only covers the 128-wide diagonal block), oT_h += v_h^T attnT and
  z row 32h += ones^T attnT accumulate in PSUM.  Per-strip normalize
  runs inside the attention loop (reciprocal + K=1 broadcast matmul)
  so the output projection starts on a hot PE.

  Output projection is weight-stationary and transposed:
  outT[dblock, :] = sum_h Wo_h[:, dblock]^T oT_h with Wo as the
  stationary operand (no per-matmul weight reloads), PSUM spanning all
  8 banks for two blocks in flight, and per-chunk ACT copies feeding
  12 contiguous [128, 2048] output DMAs.
"""

import sys

sys.path.insert(0, "/opt/trn_rl_repo")

from contextlib import ExitStack

import ml_dtypes
import numpy as np

import concourse.bass as bass
import concourse.tile as tile
from concourse import bacc, mybir
from concourse.bass_utils import run_bass_kernel_spmd

B, L, D = 2, 2048, 1536
H, FDIM, HD = 12, 16, 128
NH = 3          # heads per core
GQK = 96        # packed q+k rows (3 heads x (16 q + 16 k))
DV = NH * HD    # 384 v/o columns per core
SW = 512        # l-strip width
P = 128
NK = D // P     # 12 contraction tiles
NM = L // P     # 16 m/l tiles
NJ = L // SW    # 4 l strips
NDB = D // P    # 12 output row blocks (transposed out)

DT = mybir.dt.bfloat16
NPDT = ml_dtypes.bfloat16
F32 = mybir.dt.float32
F32R = mybir.dt.float32r

_ADD = mybir.AluOpType.add
_MULT = mybir.AluOpType.mult
_SQUARE = mybir.ActivationFunctionType.Square
_COPY = mybir.ActivationFunctionType.Copy


def _build():
    nc = bacc.Bacc("TRN2", target_bir_lowering=False, debug=False, num_devices=8)

    # host-packed inputs: contiguous blocks matching the SBUF layout
    hsp = nc.dram_tensor("hsp", [NJ * NK, P, SW], DT, kind="ExternalInput").ap()
    wqk = nc.dram_tensor("wqk", [P, NK * GQK], DT, kind="ExternalInput").ap()
    sel = nc.dram_tensor("sel", [GQK, 2 * GQK], DT, kind="ExternalInput").ap()
    wv = nc.dram_tensor("wv", [P, NK * DV], DT, kind="ExternalInput").ap()
    wo = nc.dram_tensor("wo", [P, NH * D], DT, kind="ExternalInput").ap()
    masks = nc.dram_tensor("masks", [P, 4 * SW], DT, kind="ExternalInput").ap()
    out = nc.dram_tensor("out", [NDB, P, L], DT, kind="ExternalOutput").ap()

    with tile.TileContext(nc, trace_sim=False) as tc, ExitStack() as ctx:
        cpool = ctx.enter_context(tc.tile_pool(name="consts", bufs=1))
        wqk_sb = cpool.tile([P, NK * GQK], DT, tag="wqk")
        sel_sb = cpool.tile([GQK, 2 * GQK], DT, tag="sel")
        wv_sb = cpool.tile([P, NK * DV], DT, tag="wv")
        wo_sb = cpool.tile([P, NH * D], DT, tag="wo")
        masks_sb = cpool.tile([P, 4 * SW], DT, tag="masks")
        ones_row = cpool.tile([GQK, P], mybir.dt.float16, tag="ones_row")
        warm = cpool.tile([P, SW], DT, tag="warm")
        hpool = ctx.enter_context(tc.tile_pool(name="hsT", bufs=NK))
        hs_t = [hpool.tile([P, L], DT, tag="hsT", name=f"hsT{k}") for k in range(NK)]

        # input DMAs: strip-major hsT chunks round-robin over 4 queues so
        # strip 0 lands fast; weights on scalar's queue (idle early)
        qeng = [nc.sync, nc.gpsimd, nc.scalar]
        nc.scalar.dma_start(wqk_sb[:], wqk[:])
        nc.scalar.dma_start(sel_sb[:], sel[:])
        for k in range(NK):
            qeng[k % 3].dma_start(hs_t[k][:, 0:SW], hsp[k])
        for j in range(1, NJ):
            for k in range(NK):
                qeng[(j * NK + k) % 3].dma_start(
                    hs_t[k][:, j * SW : (j + 1) * SW], hsp[j * NK + k])
        nc.scalar.dma_start(wv_sb[:], wv[:])
        nc.scalar.dma_start(masks_sb[:], masks[:])
        nc.scalar.dma_start(wo_sb[:], wo[:])
        nc.vector.memset(ones_row[:], 1.0)
        nc.vector.memset(warm[:], 0.125)

        qkv_pool = ctx.enter_context(tc.tile_pool(name="qkv", bufs=1))
        qT_sb = qkv_pool.tile([GQK, L], DT, tag="qT")
        kT_sb = qkv_pool.tile([GQK, L], DT, tag="kT")
        v_sb = qkv_pool.tile([P, NM * DV], DT, tag="v")
        qkc_pool = ctx.enter_context(tc.tile_pool(name="qkc", bufs=2))

        # ---- PE warmup during DMA priming: keep the p-state ramp going ----
        with tc.tile_pool(name="ps_warm", bufs=1, space="PSUM") as ps_warm:
            wp = ps_warm.tile([P, SW], F32, tag="wp")
            for _ in range(20):
                nc.tensor.matmul(wp[:], warm[:, 0:P], warm[:], start=True, stop=True)

        # ---- projections ----
        with tc.tile_pool(name="ps_proj", bufs=6, space="PSUM") as ps_proj:
            for j in range(NJ):
                # compact q+k chain: all 96 output rows useful
                qkp = ps_proj.tile([GQK, SW], F32, tag="p", name=f"qkp{j}")
                for k in range(NK):
                    nc.tensor.matmul(
                        qkp[:], wqk_sb[:, k * GQK : (k + 1) * GQK],
                        hs_t[k][:, j * SW : (j + 1) * SW],
                        start=(k == 0), stop=(k == NK - 1))
                qkc = qkc_pool.tile([GQK, SW], DT, tag="qkc")
                nc.vector.tensor_copy(qkc[:], qkp[:])
                # spread rows to the 32-aligned layout via selection matmuls
                spq = ps_proj.tile([GQK, SW], F32, tag="p", name=f"spq{j}")
                spk = ps_proj.tile([GQK, SW], F32, tag="p", name=f"spk{j}")
                nc.tensor.matmul(spq[:], sel_sb[:, 0:GQK], qkc[:],
                                 start=True, stop=True)
                nc.tensor.matmul(spk[:], sel_sb[:, GQK : 2 * GQK], qkc[:],
                                 start=True, stop=True)
                nc.vector.tensor_copy(qT_sb[:, j * SW : (j + 1) * SW], spq[:])
                nc.vector.tensor_copy(kT_sb[:, j * SW : (j + 1) * SW], spk[:])
            for mt in range(NM):
                vp = ps_proj.tile([P, DV], F32, tag="p")
                for k in range(NK):
                    nc.tensor.matmul(
                        vp[:], hs_t[k][:, mt * P : (mt + 1) * P],
                        wv_sb[:, k * DV : (k + 1) * DV],
                        start=(k == 0), stop=(k == NK - 1))
                nc.vector.tensor_copy(v_sb[:, mt * DV : (mt + 1) * DV], vp[:])

        opool = ctx.enter_context(tc.tile_pool(name="oT", bufs=1))
        oT_sb = [opool.tile([P, L], DT, tag=f"oT{h}", name=f"oT{h}") for h in range(NH)]
        zr32s = [opool.tile([GQK, SW], F32, tag=f"zr32_{u}", name=f"zr32_{u}")
                 for u in range(2)]
        zr16s = [opool.tile([GQK, SW], mybir.dt.float16, tag=f"zr16_{u}",
                            name=f"zr16_{u}") for u in range(2)]
        ones_col = opool.tile([P, 1], DT, tag="ones_col")
        nc.vector.memset(ones_col[:], 1.0)

        # ---- attention strips ----
        with tc.tile_pool(name="ps_sT", bufs=4, space="PSUM") as ps_sT, \
             tc.tile_pool(name="ps_oT", bufs=NH, space="PSUM") as ps_oT, \
             tc.tile_pool(name="ps_z", bufs=1, space="PSUM") as ps_z, \
             tc.tile_pool(name="attnT", bufs=28) as apool:
            for j in range(NJ):
                nim = 4 * (j + 1)
                otp = [ps_oT.tile([P, SW], F32, tag="o", name=f"otp{j}_{hh}") for hh in range(NH)]
                zp = ps_z.tile([GQK, SW], F32, tag="z")

                def emit_av(im, atts, j=j, nim=nim, otp=otp, zp=zp):
                    c = im - 4 * j
                    f0 = max(c, 0) * P  # first live column of this m-tile's strip
                    for h in range(NH):
                        nc.tensor.matmul(
                            otp[h][:, f0:SW], v_sb[:, im * DV + h * HD : im * DV + (h + 1) * HD],
                            atts[h][:, f0:SW], start=(im == 0), stop=(im == nim - 1))
                    for h in range(NH):
                        nc.tensor.matmul(
                            zp[32 * h : 32 * h + 1, f0:SW], ones_col[:], atts[h][:, f0:SW],
                            start=(im == 0), stop=(im == nim - 1))

                prev_atts = None
                for im in range(nim):
                    c = im - 4 * j
                    cur = []
                    f0 = max(c, 0) * P
                    for h in range(NH):
                        r0 = 32 * h
                        stp = ps_sT.tile([P, SW], F32, tag="s")
                        nc.tensor.matmul(
                            stp[:, f0:SW], kT_sb[r0 : r0 + FDIM, im * P : (im + 1) * P],
                            qT_sb[r0 : r0 + FDIM, j * SW + f0 : (j + 1) * SW],
                            start=True, stop=True)
                        att = apool.tile([P, SW], DT, tag="a")
                        nc.scalar.activation(att[:, f0:SW], stp[:, f0:SW], _SQUARE,
                                             bias=1.0, scale=1.0)
                        if c >= 0:
                            # mask only matters in the 128-wide diagonal block
                            nc.vector.scalar_tensor_tensor(
                                att[:, f0 : f0 + P], att[:, f0 : f0 + P], 1.0,
                                masks_sb[:, c * SW + f0 : c * SW + f0 + P],
                                op0=_ADD, op1=_MULT)
                            if f0 + P < SW:
                                nc.vector.tensor_scalar_add(
                                    att[:, f0 + P : SW], att[:, f0 + P : SW], 1.0)
                        else:
                            nc.vector.tensor_scalar_add(att[:], att[:], 1.0)
                        cur.append(att)
                    if prev_atts is not None:
                        emit_av(im - 1, prev_atts)
                    prev_atts = cur
                emit_av(nim - 1, prev_atts)

                zr32 = zr32s[j % 2]
                zr16 = zr16s[j % 2]
                nc.vector.reciprocal_approx_fast(zr32[:], zp[:])
                with nc.allow_low_precision(reason="1/z in fp16 is plenty for 2e-2 gate"):
                    nc.vector.tensor_copy(zr16[:], zr32[:])
                # normalize this strip now so the output projection starts hot
                for h in range(NH):
                    r0 = 32 * h
                    bc = ps_sT.tile([P, SW], F32, tag="s", name=f"bc{j}_{h}")
                    nc.tensor.matmul(
                        bc[:], ones_row[r0 : r0 + 1, :],
                        zr16[r0 : r0 + 1, :],
                        start=True, stop=True)
                    nc.vector.tensor_copy(oT_sb[h][:, j * SW : (j + 1) * SW], otp[h][:])
                    nc.vector.tensor_mul(
                        oT_sb[h][:, j * SW : (j + 1) * SW],
                        oT_sb[h][:, j * SW : (j + 1) * SW], bc[:])

        # ---- transposed, weight-stationary output projection ----
        with tc.tile_pool(name="ps_out", bufs=8, space="PSUM") as ps_out, \
             tc.tile_pool(name="obuf", bufs=3) as obuf:
            for db in range(NDB):
                ops = [ps_out.tile([P, SW], F32, tag="op", name=f"op{db}_{jc}")
                       for jc in range(NJ)]
                ob = obuf.tile([P, L], DT, tag="ob")
                for h in range(NH):
                    for jc in range(NJ):
                        nc.tensor.matmul(
                            ops[jc][:], wo_sb[:, h * D + db * P : h * D + (db + 1) * P],
                            oT_sb[h][:, jc * SW : (jc + 1) * SW],
                            start=(h == 0), stop=(h == NH - 1))
                        if h == NH - 1:
                            # copy each chunk as soon as its chain stops
                            nc.scalar.activation(
                                ob[:, jc * SW : (jc + 1) * SW], ops[jc][:], _COPY)
                (nc.gpsimd if db % 2 else nc.sync).dma_start(out[db], ob[:])

    nc.compile()
    return nc


def _host_inputs(hidden_states, Wq, Wk, Wv, Wo):
    """Shard + lay out the full inputs into 8 per-core in_maps."""
    scale = FDIM ** -0.5
    mask = np.zeros((P, 4 * SW), dtype=np.float32)
    for c in range(4):
        p = np.arange(P)[:, None] + 128 * c
        f = np.arange(SW)[None, :]
        mask[:, c * SW : (c + 1) * SW] = (p <= f).astype(np.float32)

    in_maps = []
    for core in range(8):
        b, g = divmod(core, 4)
        heads = range(NH * g, NH * (g + 1))
        # wqk compact: per k-chunk [q0 q1 q2 k0 k1 k2] x 16 cols
        wqk_pad = np.zeros((P, NK * GQK), dtype=np.float32)
        for k in range(NK):
            c0 = k * GQK
            for i, h in enumerate(heads):
                wqk_pad[:, c0 + 16 * i : c0 + 16 * i + 16] = \
                    Wq[k * P : (k + 1) * P, FDIM * h : FDIM * (h + 1)] * scale
                wqk_pad[:, c0 + 48 + 16 * i : c0 + 48 + 16 * i + 16] = \
                    Wk[k * P : (k + 1) * P, FDIM * h : FDIM * (h + 1)]
        # selection matrices: row r=16i+f -> out row 32i+f (q), ditto k
        selm = np.zeros((GQK, 2 * GQK), dtype=np.float32)
        for i in range(NH):
            for f in range(16):
                selm[16 * i + f, 32 * i + f] = 1.0
                selm[48 + 16 * i + f, GQK + 32 * i + f] = 1.0
        # wv packed [128, 12*384]; wo packed [128, 3*1536]
        wv_l = Wv[:, HD * NH * g : HD * NH * (g + 1)]
        wv_pad = np.concatenate(
            [wv_l[k * P : (k + 1) * P, :] for k in range(NK)], axis=1)
        wo_l = Wo[HD * NH * g : HD * NH * (g + 1), :]
        wo_pad = np.concatenate(
            [wo_l[h * P : (h + 1) * P, :] for h in range(NH)], axis=1)
        # hsT strip-major contiguous chunks [j*NK+k] = hsT[128k:128k+128, 512j:...]
        hsT = np.ascontiguousarray(hidden_states[b].T).astype(NPDT)
        hsp = np.empty((NJ * NK, P, SW), dtype=NPDT)
        for j in range(NJ):
            for k in range(NK):
                hsp[j * NK + k] = hsT[k * P : (k + 1) * P, j * SW : (j + 1) * SW]
        in_maps.append({
            "hsp": hsp,
            "wqk": wqk_pad.astype(NPDT),
            "wv": np.ascontiguousarray(wv_pad).astype(NPDT),
            "wo": np.ascontiguousarray(wo_pad).astype(NPDT),
            "masks": mask.astype(NPDT),
            "sel": selm.astype(NPDT),
        })
    return in_maps


_NC = None


def _get_nc():
    global _NC
    if _NC is None:
        _NC = _build()
    return _NC


def run(hidden_states, Wq, Wk, Wv, Wo, trace=False, **trace_kwargs):
    nc = _get_nc()
    in_maps = _host_inputs(hidden_states, Wq, Wk, Wv, Wo)
    res = run_bass_kernel_spmd(nc, in_maps, core_ids=list(range(8)),
                               trace=trace, **trace_kwargs)
    out = np.zeros((B, L, D), dtype=np.float32)
    for core in range(8):
        outT = res.results[core]["out"].astype(np.float32)  # [12, 128, 2048]
        out[core // 4] += outT.reshape(D, L).T
    return out, res


def kernel(hidden_states, Wq, Wk, Wv, Wo):
    out, _ = run(np.asarray(hidden_states, dtype=np.float32),
                 np.asarray(Wq, dtype=np.float32),
                 np.asarray(Wk, dtype=np.float32),
                 np.asarray(Wv, dtype=np.float32),
                 np.asarray(Wo, dtype=np.float32))
    return out


if __name__ == "__main__":
    # quick numpy self-check of host packing against the reference math
    rng = np.random.default_rng(0)
    hs = rng.normal(size=(B, L, D)).astype(np.float32)
    Wq = (rng.normal(size=(D, H * FDIM)) * 0.02).astype(np.float32)
    Wk = (rng.normal(size=(D, H * FDIM)) * 0.02).astype(np.float32)
    Wv = (rng.normal(size=(D, H * HD)) * 0.02).astype(np.float32)
    Wo = (rng.normal(size=(H * HD, D)) * 0.02).astype(np.float32)
    out = kernel(hs, Wq, Wk, Wv, Wo)
    print(out.shape)
